# revision 6
# baseline (speedup 1.0000x reference)
"""AdditiveAttention Trainium2 kernel (8 NeuronCores, data-parallel over batch).

Math: scores[b,q,k] = sum_h wv[h] * tanh(qp[b,q,h] + kp[b,k,h]) with
qp = queries @ Wq^T, kp = keys @ Wk^T, then length-masked softmax over k and
attn @ values.

Device strategy (per core, 2 batch slots):
  tanh(x) ~= sum_t c_t sin(w_t x), w_t = (2t-1)*w0 (odd harmonics, fit under
  the N(0,2) distribution of qp+kp). sin(w(a+b)) = sin(wa)cos(wb) +
  cos(wa)sin(wb) turns scoring into matmuls with contraction 2*T*H.
  All harmonics come from one in-domain ACT Sin pair via the odd-step
  Chebyshev recurrence f_{k+2} = 2cos(2w0 x) f_k - f_{k-2}, computed on
  [qp;kp]-stacked tiles with sin|cos halves merged so each step is 2 DVE ops.
  Softmax needs no max pass (scores are bounded); the 0/1 length mask and the
  ones-column that produces Z are folded into V on the host, so softmax is
  exp -> AV-matmul -> scale by 1/Z. Inputs arrive as one packed per-partition
  blob -> few large contiguous DMAs.
"""

import os
import sys

for _p in ("/opt/trn_rl_repo", os.path.expanduser("~/.axon_site/_ro/trn_rl_repo")):
    if os.path.isdir(_p) and _p not in sys.path:
        sys.path.insert(0, _p)

import math

import ml_dtypes
import numpy as np

import concourse.bass as bass
import concourse.mybir as mybir
import concourse.tile as tile
from concourse import bacc
from concourse.bass_utils import run_bass_kernel_spmd

BF16 = ml_dtypes.bfloat16
F32 = mybir.dt.float32
BF = mybir.dt.bfloat16

B, Q, K, H = 16, 512, 512, 64
DQ = DK = DV = 256
P = 128
NCORES = 8
SLOTS = 2

W0 = 0.4010
CS = np.array([1.193248, 0.247628, 0.069403, 0.018763, 0.006754], np.float64)
T = len(CS)

AF = mybir.ActivationFunctionType
ALU = mybir.AluOpType

_COMPILE_CACHE = {}

TRACE = False
LAST_RESULTS = None


def _offsets(kt_bounds):
    """Per-partition element offsets inside the packed bf16 blob."""
    off = {}
    o = 0
    off["wq"] = o
    o += 2 * H
    off["wk"] = o
    o += 2 * H
    for s in range(SLOTS):
        off[f"q{s}"] = o
        o += 2 * Q
        off[f"k{s}"] = o
        o += 2 * K
    for s in range(SLOTS):
        off[f"v{s}"] = o
        o += (DV + 1) * kt_bounds[s]
    off["end"] = o
    return off


def _build(kt_bounds):
    nc = bacc.Bacc()
    off = _offsets(kt_bounds)
    XB = off["end"]

    ib = nc.declare_dram_parameter("ib", [P, XB], BF, isOutput=False)
    cwv = nc.declare_dram_parameter("cwv", [P, T], F32, isOutput=False)
    out = nc.declare_dram_parameter("out", [SLOTS, Q, DV], F32, isOutput=True)

    with tile.TileContext(nc) as tc:
        with (
            tc.tile_pool(name="singles", bufs=1) as singles,
            tc.tile_pool(name="lad", bufs=2) as lad,
            tc.tile_pool(name="feat", bufs=2) as feat,
            tc.tile_pool(name="esb", bufs=2) as esb,
            tc.tile_pool(name="osb", bufs=4) as osb,
            tc.tile_pool(name="pproj", bufs=2, space="PSUM") as pproj,
            tc.tile_pool(name="psc", bufs=2, space="PSUM") as psc,
            tc.tile_pool(name="pav", bufs=2, space="PSUM") as pav,
        ):
            ib_sb = singles.tile([P, XB], BF)
            # input blob in four slices across independent DMA queues
            cut1 = off["q1"]          # weights + slot0 q/k
            cut2 = off["v0"]          # slot1 q/k
            cut3 = off["v1"]          # slot0 vaug
            nc.sync.dma_start(ib_sb[:, 0:cut1], ib[:, 0:cut1])
            nc.scalar.dma_start(ib_sb[:, cut1:cut2], ib[:, cut1:cut2])
            nc.gpsimd.dma_start(ib_sb[:, cut2:cut3], ib[:, cut2:cut3])
            nc.sync.dma_start(ib_sb[:, cut3:XB], ib[:, cut3:XB])
            cwv_sb = singles.tile([P, T], F32)
            nc.sync.dma_start(cwv_sb[:], cwv[:, :])

            wq_v = ib_sb[:, off["wq"] : off["wq"] + 2 * H].rearrange(
                "p (c h) -> p c h", c=2
            )
            wk_v = ib_sb[:, off["wk"] : off["wk"] + 2 * H].rearrange(
                "p (c h) -> p c h", c=2
            )

            fA = [None] * SLOTS
            fB = [None] * SLOTS
            va_v = [None] * SLOTS

            # ---------------- phase A: proj + harmonic ladder + features ----
            for s in range(SLOTS):
                ktn = kt_bounds[s]
                q_v = ib_sb[:, off[f"q{s}"] : off[f"q{s}"] + 2 * Q].rearrange(
                    "p (c q) -> p c q", c=2
                )
                k_v = ib_sb[:, off[f"k{s}"] : off[f"k{s}"] + 2 * K].rearrange(
                    "p (c k) -> p c k", c=2
                )
                va_v[s] = ib_sb[
                    :, off[f"v{s}"] : off[f"v{s}"] + (DV + 1) * ktn
                ].rearrange("p (kt v) -> p kt v", kt=ktn)

                # stacked projection qk = [qp; kp] in one PSUM bank
                qk = pproj.tile([P, Q], F32, tag="qk")
                for c in range(2):
                    nc.tensor.matmul(
                        qk[0:H, :], wq_v[:, c, :], q_v[:, c, :],
                        start=(c == 0), stop=(c == 1), tile_position=(0, 0),
                    )
                for c in range(2):
                    nc.tensor.matmul(
                        qk[H:P, :], wk_v[:, c, :], k_v[:, c, :],
                        start=(c == 0), stop=(c == 1), tile_position=(0, H),
                    )

                # LAD[:, t, 0:512] = sin((2t+1) w0 x), [:, t, 512:1024] = cos
                LD = lad.tile([P, T, 2 * Q], BF, tag="LD")
                sh_t = lad.tile([P, Q], BF, tag="sh_t")
                sq1 = lad.tile([P, Q], BF, tag="sq1")
                sqh = lad.tile([P, Q], BF, tag="sqh")
                c2 = lad.tile([P, Q], BF, tag="c2")

                nc.scalar.activation(LD[:, 0, 0:Q], qk[:], AF.Sin, scale=W0)
                nc.scalar.activation(sh_t[:], qk[:], AF.Sin, scale=W0 / 2.0)
                nc.scalar.activation(sq1[:], LD[:, 0, 0:Q], AF.Square)
                nc.scalar.activation(sqh[:], sh_t[:], AF.Square)
                # c2 = 2cos(2 w0 x);  LAD cos_1 = 1 - 2 sin^2(w0 x / 2)
                nc.vector.tensor_scalar(c2[:], sq1[:], -4.0, 2.0, ALU.mult, ALU.add)
                nc.vector.tensor_scalar(
                    LD[:, 0, Q : 2 * Q], sqh[:], -2.0, 1.0, ALU.mult, ALU.add
                )

                # broadcast view of c2 over the sin|cos halves
                c2ap = c2[:]
                c2b = bass.AP(
                    tensor=c2ap.tensor,
                    offset=c2ap.offset,
                    ap=[c2ap.ap[0], [0, 2], c2ap.ap[1]],
                )

                for t in range(1, T):
                    tmp = lad.tile([P, 2 * Q], BF, tag="ltmp")
                    nc.vector.tensor_tensor(tmp[:], c2b, LD[:, t - 1, :], ALU.mult)
                    if t == 1:
                        nc.vector.tensor_tensor(
                            LD[:, 1, 0:Q], tmp[:, 0:Q], LD[:, 0, 0:Q], ALU.add
                        )
                        nc.vector.tensor_tensor(
                            LD[:, 1, Q : 2 * Q], tmp[:, Q : 2 * Q],
                            LD[:, 0, Q : 2 * Q], ALU.subtract,
                        )
                    else:
                        nc.vector.tensor_tensor(
                            LD[:, t, :], tmp[:], LD[:, t - 2, :], ALU.subtract
                        )

                # feature chunks: fA_t = [sinA_t; cosA_t], fB_t = [cosB_t;
                # sinB_t]*cwv_t.  A parts live in partitions 0:64 of LAD, B
                # parts in 64:128; four bulk DMAs do the partition moves.
                fa = feat.tile([P, T, Q], BF, tag="fa")
                fb = feat.tile([P, T, Q], BF, tag="fb")
                nc.gpsimd.dma_start(fa[0:H, :, :], LD[0:H, :, 0:Q])
                nc.sync.dma_start(fa[H:P, :, :], LD[0:H, :, Q : 2 * Q])
                nc.scalar.dma_start(fb[0:H, :, :], LD[H:P, :, Q : 2 * Q])
                nc.gpsimd.dma_start(fb[H:P, :, :], LD[H:P, :, 0:Q])
                for t in range(T):
                    nc.gpsimd.tensor_scalar_mul(
                        fb[:, t, :], fb[:, t, :], cwv_sb[:, t : t + 1]
                    )
                fA[s], fB[s] = fa, fb

            # ---------------- phase B: scores, softmax, AV, out -------------
            for s in range(SLOTS):
                ktn = kt_bounds[s]
                e_tiles = []
                for kt in range(ktn):
                    sc = psc.tile([P, Q], F32, tag="sc")
                    for t in range(T):
                        nc.tensor.matmul(
                            sc[:],
                            fB[s][:, t, kt * P : (kt + 1) * P],
                            fA[s][:, t, :],
                            start=(t == 0),
                            stop=(t == T - 1),
                        )
                    e_kt = esb.tile([P, Q], BF, tag=f"e{kt}")
                    nc.scalar.activation(e_kt[:], sc[:], AF.Exp)
                    e_tiles.append(e_kt)

                for qt in range(Q // P):
                    o_ps = pav.tile([P, DV + 1], F32, tag="o_ps")
                    for kt in range(ktn):
                        nc.tensor.matmul(
                            o_ps[:],
                            e_tiles[kt][:, qt * P : (qt + 1) * P],
                            va_v[s][:, kt, :],
                            start=(kt == 0),
                            stop=(kt == ktn - 1),
                        )
                    rz = osb.tile([P, 1], F32, tag="rz")
                    nc.vector.reciprocal(rz[:], o_ps[:, DV : DV + 1])
                    o_sb = osb.tile([P, DV], F32, tag="o_sb")
                    nc.vector.tensor_scalar_mul(o_sb[:], o_ps[:, 0:DV], rz[:])
                    eng = nc.sync if qt % 2 == 0 else nc.scalar
                    eng.dma_start(out[s, qt * P : (qt + 1) * P, :], o_sb[:])

    nc.finalize()
    return nc


def kernel(queries, keys, values, valid_lens, Wq, Wk, wv):
    global LAST_RESULTS
    queries = np.asarray(queries, np.float32)
    keys = np.asarray(keys, np.float32)
    values = np.asarray(values, np.float32)
    vl = np.asarray(valid_lens).astype(np.int64)
    Wq = np.asarray(Wq, np.float32)
    Wk = np.asarray(Wk, np.float32)
    wv = np.asarray(wv, np.float32)

    order = np.argsort(-vl, kind="stable")
    slot_b = [order[:NCORES], order[NCORES:]]
    kt_bounds = tuple(max(1, math.ceil(int(vl[sb].max()) / P)) for sb in slot_b)

    if kt_bounds not in _COMPILE_CACHE:
        _COMPILE_CACHE[kt_bounds] = _build(kt_bounds)
    nc = _COMPILE_CACHE[kt_bounds]
    off = _offsets(kt_bounds)
    XB = off["end"]

    # host-side packing --------------------------------------------------
    def chunked(mat, d_in, width):
        # [d_in, width] -> [128, nchunks*width] with chunk-major per partition
        n = d_in // P
        return (
            mat.reshape(n, P, width).transpose(1, 0, 2).reshape(P, n * width)
        )

    mask = (np.arange(K)[None, :] < vl[:, None]).astype(np.float32)  # [B, K]
    vaug = np.concatenate(
        [values * mask[:, :, None], mask[:, :, None]], axis=2
    )  # [B, K, 257]

    qT = np.ascontiguousarray(queries.transpose(0, 2, 1))  # [B, 256, 512]
    kT = np.ascontiguousarray(keys.transpose(0, 2, 1))

    wq_p = chunked(np.ascontiguousarray(Wq.T), DQ, H)  # [128, 128]
    wk_p = chunked(np.ascontiguousarray(Wk.T), DK, H)

    blobs = np.empty((NCORES, P, XB), BF16)
    for i in range(NCORES):
        for s in range(SLOTS):
            b = int(slot_b[s][i])
            ktn = kt_bounds[s]
            blobs[i, :, off[f"q{s}"] : off[f"q{s}"] + 2 * Q] = chunked(
                qT[b], DQ, Q
            )
            blobs[i, :, off[f"k{s}"] : off[f"k{s}"] + 2 * K] = chunked(
                kT[b], DK, K
            )
            blobs[i, :, off[f"v{s}"] : off[f"v{s}"] + (DV + 1) * ktn] = (
                vaug[b, : ktn * P]
                .reshape(ktn, P, DV + 1)
                .transpose(1, 0, 2)
                .reshape(P, ktn * (DV + 1))
            )
        blobs[i, :, off["wq"] : off["wq"] + 2 * H] = wq_p
        blobs[i, :, off["wk"] : off["wk"] + 2 * H] = wk_p

    cwv_h = (CS[None, :] * wv[:, None].astype(np.float64)).astype(np.float32)
    cwv_full = np.concatenate([cwv_h, cwv_h], axis=0)  # [128, T]

    in_maps = [{"ib": blobs[i], "cwv": cwv_full} for i in range(NCORES)]

    res = run_bass_kernel_spmd(
        nc, in_maps, core_ids=list(range(NCORES)), trace=TRACE
    )
    LAST_RESULTS = res

    out = np.empty((B, Q, DV), np.float32)
    for i in range(NCORES):
        o = np.asarray(res.results[i]["out"])
        out[slot_b[0][i]] = o[0]
        out[slot_b[1][i]] = o[1]
    return out


# revision 7
# speedup vs baseline: 2.2583x; 2.2583x over previous
"""AdditiveAttention Trainium2 kernel (8 NeuronCores, data-parallel over batch).

Math: scores[b,q,k] = sum_h wv[h] * tanh(qp[b,q,h] + kp[b,k,h]) with
qp = queries @ Wq^T, kp = keys @ Wk^T, then length-masked softmax over k and
attn @ values.

Device strategy (per core, 2 batch slots):
  tanh(x) ~= sum_t c_t sin(w_t x), w_t = (2t-1)*w0 (odd harmonics, fit under
  the N(0,2) distribution of qp+kp). sin(w(a+b)) = sin(wa)cos(wb) +
  cos(wa)sin(wb) turns scoring into matmuls with contraction 2*T*H.
  All harmonics come from one in-domain ACT Sin pair via the odd-step
  Chebyshev recurrence f_{k+2} = 2cos(2w0 x) f_k - f_{k-2}, computed on
  [qp;kp]-stacked tiles with sin|cos halves merged so each step is 2 DVE ops.
  Softmax needs no max pass (scores are bounded); the 0/1 length mask and the
  ones-column that produces Z are folded into V on the host, so softmax is
  exp -> AV-matmul -> scale by 1/Z. Inputs arrive as one packed per-partition
  blob -> few large contiguous DMAs.
"""

import os
import sys

for _p in ("/opt/trn_rl_repo", os.path.expanduser("~/.axon_site/_ro/trn_rl_repo")):
    if os.path.isdir(_p) and _p not in sys.path:
        sys.path.insert(0, _p)

import math

import ml_dtypes
import numpy as np

import concourse.bass as bass
import concourse.mybir as mybir
import concourse.tile as tile
from concourse import bacc
from concourse.bass_utils import run_bass_kernel_spmd

BF16 = ml_dtypes.bfloat16
F32 = mybir.dt.float32
BF = mybir.dt.bfloat16

B, Q, K, H = 16, 512, 512, 64
DQ = DK = DV = 256
P = 128
NCORES = 8
SLOTS = 2

W0 = 0.4010
CS = np.array([1.193248, 0.247628, 0.069403, 0.018763, 0.006754], np.float64)
T = len(CS)

AF = mybir.ActivationFunctionType
ALU = mybir.AluOpType

_COMPILE_CACHE = {}

TRACE = False
LAST_RESULTS = None


def _offsets(kt_bounds):
    """Per-partition element offsets inside the packed bf16 blob."""
    off = {}
    o = 0
    off["wq"] = o
    o += 2 * H
    off["wk"] = o
    o += 2 * H
    for s in range(SLOTS):
        off[f"q{s}"] = o
        o += 2 * Q
        off[f"k{s}"] = o
        o += 2 * K
    for s in range(SLOTS):
        off[f"v{s}"] = o
        o += (DV + 1) * kt_bounds[s]
    off["end"] = o
    return off


def _build(kt_bounds):
    nc = bacc.Bacc()
    off = _offsets(kt_bounds)
    XB = off["end"]

    ib = nc.declare_dram_parameter("ib", [P, XB], BF, isOutput=False)
    cwv = nc.declare_dram_parameter("cwv", [P, T], F32, isOutput=False)
    out = nc.declare_dram_parameter("out", [SLOTS, Q, DV], F32, isOutput=True)

    with tile.TileContext(nc) as tc:
        with (
            tc.tile_pool(name="singles", bufs=1) as singles,
            tc.tile_pool(name="lad", bufs=2) as lad,
            tc.tile_pool(name="feat", bufs=2) as feat,
            tc.tile_pool(name="esb", bufs=2) as esb,
            tc.tile_pool(name="osb", bufs=4) as osb,
            tc.tile_pool(name="pproj", bufs=2, space="PSUM") as pproj,
            tc.tile_pool(name="psc", bufs=2, space="PSUM") as psc,
            tc.tile_pool(name="pav", bufs=2, space="PSUM") as pav,
        ):
            ib_sb = singles.tile([P, XB], BF)
            # input blob in four slices across independent DMA queues
            cut1 = off["q1"]          # weights + slot0 q/k
            cut2 = off["v0"]          # slot1 q/k
            cut3 = off["v1"]          # slot0 vaug
            nc.sync.dma_start(ib_sb[:, 0:cut1], ib[:, 0:cut1])
            nc.scalar.dma_start(ib_sb[:, cut1:cut2], ib[:, cut1:cut2])
            nc.sync.dma_start(ib_sb[:, cut2:cut3], ib[:, cut2:cut3])
            nc.scalar.dma_start(ib_sb[:, cut3:XB], ib[:, cut3:XB])
            cwv_sb = singles.tile([P, T], F32)
            nc.sync.dma_start(cwv_sb[:], cwv[:, :])

            wq_v = ib_sb[:, off["wq"] : off["wq"] + 2 * H].rearrange(
                "p (c h) -> p c h", c=2
            )
            wk_v = ib_sb[:, off["wk"] : off["wk"] + 2 * H].rearrange(
                "p (c h) -> p c h", c=2
            )

            fA = [None] * SLOTS
            fB = [None] * SLOTS
            va_v = [None] * SLOTS

            # ---------------- phase A: proj + harmonic ladder + features ----
            for s in range(SLOTS):
                ktn = kt_bounds[s]
                q_v = ib_sb[:, off[f"q{s}"] : off[f"q{s}"] + 2 * Q].rearrange(
                    "p (c q) -> p c q", c=2
                )
                k_v = ib_sb[:, off[f"k{s}"] : off[f"k{s}"] + 2 * K].rearrange(
                    "p (c k) -> p c k", c=2
                )
                va_v[s] = ib_sb[
                    :, off[f"v{s}"] : off[f"v{s}"] + (DV + 1) * ktn
                ].rearrange("p (kt v) -> p kt v", kt=ktn)

                # stacked projection qk = [qp; kp] in one PSUM bank
                qk = pproj.tile([P, Q], F32, tag="qk")
                for c in range(2):
                    nc.tensor.matmul(
                        qk[0:H, :], wq_v[:, c, :], q_v[:, c, :],
                        start=(c == 0), stop=(c == 1), tile_position=(0, 0),
                    )
                for c in range(2):
                    nc.tensor.matmul(
                        qk[H:P, :], wk_v[:, c, :], k_v[:, c, :],
                        start=(c == 0), stop=(c == 1), tile_position=(0, H),
                    )

                # LAD[:, t, 0:512] = sin((2t+1) w0 x), [:, t, 512:1024] = cos
                LD = lad.tile([P, T, 2 * Q], BF, tag="LD")
                sh_t = lad.tile([P, Q], BF, tag="sh_t")
                sq1 = lad.tile([P, Q], BF, tag="sq1")
                sqh = lad.tile([P, Q], BF, tag="sqh")
                c2 = lad.tile([P, Q], BF, tag="c2")

                nc.scalar.activation(LD[:, 0, 0:Q], qk[:], AF.Sin, scale=W0)
                nc.scalar.activation(sh_t[:], qk[:], AF.Sin, scale=W0 / 2.0)
                nc.scalar.activation(sq1[:], LD[:, 0, 0:Q], AF.Square)
                nc.scalar.activation(sqh[:], sh_t[:], AF.Square)
                # c2 = 2cos(2 w0 x);  LAD cos_1 = 1 - 2 sin^2(w0 x / 2)
                nc.vector.tensor_scalar(c2[:], sq1[:], -4.0, 2.0, ALU.mult, ALU.add)
                nc.vector.tensor_scalar(
                    LD[:, 0, Q : 2 * Q], sqh[:], -2.0, 1.0, ALU.mult, ALU.add
                )

                # broadcast view of c2 over the sin|cos halves
                c2ap = c2[:]
                c2b = bass.AP(
                    tensor=c2ap.tensor,
                    offset=c2ap.offset,
                    ap=[c2ap.ap[0], [0, 2], c2ap.ap[1]],
                )

                for t in range(1, T):
                    tmp = lad.tile([P, 2 * Q], BF, tag="ltmp")
                    nc.vector.tensor_tensor(tmp[:], c2b, LD[:, t - 1, :], ALU.mult)
                    if t == 1:
                        nc.vector.tensor_tensor(
                            LD[:, 1, 0:Q], tmp[:, 0:Q], LD[:, 0, 0:Q], ALU.add
                        )
                        nc.vector.tensor_tensor(
                            LD[:, 1, Q : 2 * Q], tmp[:, Q : 2 * Q],
                            LD[:, 0, Q : 2 * Q], ALU.subtract,
                        )
                    else:
                        nc.vector.tensor_tensor(
                            LD[:, t, :], tmp[:], LD[:, t - 2, :], ALU.subtract
                        )

                # feature chunks: fA_t = [sinA_t; cosA_t], fB_t = [cosB_t;
                # sinB_t]*cwv_t.  A parts live in partitions 0:64 of LAD, B
                # parts in 64:128; four bulk DMAs do the partition moves.
                fa = feat.tile([P, T, Q], BF, tag="fa")
                fb = feat.tile([P, T, Q], BF, tag="fb")
                nc.sync.dma_start(fa[0:H, :, :], LD[0:H, :, 0:Q])
                nc.scalar.dma_start(fa[H:P, :, :], LD[0:H, :, Q : 2 * Q])
                nc.scalar.dma_start(fb[0:H, :, :], LD[H:P, :, Q : 2 * Q])
                nc.sync.dma_start(fb[H:P, :, :], LD[H:P, :, 0:Q])
                for t in range(T):
                    nc.vector.tensor_scalar_mul(
                        fb[:, t, :], fb[:, t, :], cwv_sb[:, t : t + 1]
                    )
                fA[s], fB[s] = fa, fb

            # ---------------- phase B: scores, softmax, AV, out -------------
            for s in range(SLOTS):
                ktn = kt_bounds[s]
                e_tiles = []
                for kt in range(ktn):
                    sc = psc.tile([P, Q], F32, tag="sc")
                    for t in range(T):
                        nc.tensor.matmul(
                            sc[:],
                            fB[s][:, t, kt * P : (kt + 1) * P],
                            fA[s][:, t, :],
                            start=(t == 0),
                            stop=(t == T - 1),
                        )
                    e_kt = esb.tile([P, Q], BF, tag=f"e{kt}")
                    nc.scalar.activation(e_kt[:], sc[:], AF.Exp)
                    e_tiles.append(e_kt)

                for qt in range(Q // P):
                    o_ps = pav.tile([P, DV + 1], F32, tag="o_ps")
                    for kt in range(ktn):
                        nc.tensor.matmul(
                            o_ps[:],
                            e_tiles[kt][:, qt * P : (qt + 1) * P],
                            va_v[s][:, kt, :],
                            start=(kt == 0),
                            stop=(kt == ktn - 1),
                        )
                    rz = osb.tile([P, 1], F32, tag="rz")
                    nc.vector.reciprocal(rz[:], o_ps[:, DV : DV + 1])
                    o_sb = osb.tile([P, DV], F32, tag="o_sb")
                    nc.vector.tensor_scalar_mul(o_sb[:], o_ps[:, 0:DV], rz[:])
                    eng = nc.sync if qt % 2 == 0 else nc.scalar
                    eng.dma_start(out[s, qt * P : (qt + 1) * P, :], o_sb[:])

    nc.finalize()
    return nc


def kernel(queries, keys, values, valid_lens, Wq, Wk, wv):
    global LAST_RESULTS
    queries = np.asarray(queries, np.float32)
    keys = np.asarray(keys, np.float32)
    values = np.asarray(values, np.float32)
    vl = np.asarray(valid_lens).astype(np.int64)
    Wq = np.asarray(Wq, np.float32)
    Wk = np.asarray(Wk, np.float32)
    wv = np.asarray(wv, np.float32)

    order = np.argsort(-vl, kind="stable")
    slot_b = [order[:NCORES], order[NCORES:]]
    kt_bounds = tuple(max(1, math.ceil(int(vl[sb].max()) / P)) for sb in slot_b)

    if kt_bounds not in _COMPILE_CACHE:
        _COMPILE_CACHE[kt_bounds] = _build(kt_bounds)
    nc = _COMPILE_CACHE[kt_bounds]
    off = _offsets(kt_bounds)
    XB = off["end"]

    # host-side packing --------------------------------------------------
    def chunked(mat, d_in, width):
        # [d_in, width] -> [128, nchunks*width] with chunk-major per partition
        n = d_in // P
        return (
            mat.reshape(n, P, width).transpose(1, 0, 2).reshape(P, n * width)
        )

    mask = (np.arange(K)[None, :] < vl[:, None]).astype(np.float32)  # [B, K]
    vaug = np.concatenate(
        [values * mask[:, :, None], mask[:, :, None]], axis=2
    )  # [B, K, 257]

    qT = np.ascontiguousarray(queries.transpose(0, 2, 1))  # [B, 256, 512]
    kT = np.ascontiguousarray(keys.transpose(0, 2, 1))

    wq_p = chunked(np.ascontiguousarray(Wq.T), DQ, H)  # [128, 128]
    wk_p = chunked(np.ascontiguousarray(Wk.T), DK, H)

    blobs = np.empty((NCORES, P, XB), BF16)
    for i in range(NCORES):
        for s in range(SLOTS):
            b = int(slot_b[s][i])
            ktn = kt_bounds[s]
            blobs[i, :, off[f"q{s}"] : off[f"q{s}"] + 2 * Q] = chunked(
                qT[b], DQ, Q
            )
            blobs[i, :, off[f"k{s}"] : off[f"k{s}"] + 2 * K] = chunked(
                kT[b], DK, K
            )
            blobs[i, :, off[f"v{s}"] : off[f"v{s}"] + (DV + 1) * ktn] = (
                vaug[b, : ktn * P]
                .reshape(ktn, P, DV + 1)
                .transpose(1, 0, 2)
                .reshape(P, ktn * (DV + 1))
            )
        blobs[i, :, off["wq"] : off["wq"] + 2 * H] = wq_p
        blobs[i, :, off["wk"] : off["wk"] + 2 * H] = wk_p

    cwv_h = (CS[None, :] * wv[:, None].astype(np.float64)).astype(np.float32)
    cwv_full = np.concatenate([cwv_h, cwv_h], axis=0)  # [128, T]

    in_maps = [{"ib": blobs[i], "cwv": cwv_full} for i in range(NCORES)]

    res = run_bass_kernel_spmd(
        nc, in_maps, core_ids=list(range(NCORES)), trace=TRACE
    )
    LAST_RESULTS = res

    out = np.empty((B, Q, DV), np.float32)
    for i in range(NCORES):
        o = np.asarray(res.results[i]["out"])
        out[slot_b[0][i]] = o[0]
        out[slot_b[1][i]] = o[1]
    return out
